# revision 1
# baseline (speedup 1.0000x reference)
"""Graph-LSTM (GsGLstm) Trainium2 kernel.

Strategy (B=8 -> one sample per NeuronCore, pure data parallel):
  - Host: neighbor gathers are converted to dense transposed adjacency
    matmuls  h_aggT = h^T-free PE matmul with A_T[m,n] = sum_k mask[n,k]*[idx[n,k]==m]
    (rows of masked source nodes zeroed, so no device-side node masking needed;
    final output is masked on host).
  - Host: the layer-invariant x-side preactivation pre_x = x_in@W_in + x_out@W_out + b
    is precomputed (gate-major columns) and shipped once.
  - Device per layer:  gather matmuls (stationary = h natural bf16, moving = A_T bf16)
    -> h_inT/h_outT [d, n] -> U matmuls (stationary = h_inT tiles, moving = U_cat bf16)
    -> pre natural [n, 4*256] in PSUM -> +pre_x (DVE) -> sigmoid/tanh (ACT)
    -> c/h elementwise updates (DVE).  No transposes needed anywhere.
"""

import numpy as np
import ml_dtypes

B, N, K, D = 8, 1024, 16, 256
NT = N // 128   # 8 node partition-tiles
DT = D // 128   # 2 feature partition-tiles

_CACHE = {}


def _patch_tile_drain():
    """walrus CTRL instructions have 2 sync-wait slots; TileContext's final
    drain can carry more and fails codegen. Split excess waits onto SP nops."""
    import concourse.tile as _tile

    if getattr(_tile.TileContext, "_ant_drain_patched", False):
        return
    ScopedClock = _tile.ScopedClock

    def _split_excess_waits(nc):
        import concourse.mybir as _mybir

        for f in nc.m.functions:
            for blk in f.blocks:
                insts = blk.instructions
                i = 0
                while i < len(insts):
                    ins = insts[i]
                    si = getattr(ins, "sync_info", None)
                    keep = 1
                    if si and si.on_wait and len(si.on_wait) > keep:
                        waits = list(si.on_wait)
                        head, tail = waits[:-keep], waits[-keep:]
                        si.on_wait.clear()
                        for w in tail:
                            si.on_wait.append(w)
                        eng = nc.engines[ins.engine]
                        pos = i
                        for w in head:
                            n = eng.nop(nofuse=True)
                            cur_list = nc.cur_bb.bb.instructions
                            assert cur_list[-1] is n.ins
                            cur_list.pop()
                            if n.ins.sync_info is None:
                                n.ins.sync_info = _mybir.SyncInfo(
                                    on_wait=[], on_update=[]
                                )
                            n.ins.sync_info.on_wait.append(w)
                            insts.insert(pos, n.ins)
                            pos += 1
                            i += 1
                    i += 1

    def _patched(self, tick_clock, wait_clock):
        drain_inst = self.nc.sync.drain()
        wait_clock.add_sem_waits(
            drain_inst.ins, ScopedClock({None: tick_clock.global_clock})
        )
        _split_excess_waits(self.nc)
        self.nc.all_engine_barrier()
        assert self.sems is not None
        popped = self.nc._tile_sem_poison_stack.pop()
        assert popped is self._sem_poison
        self.nc.clear_and_free_semaphores(list(self.sems.allocated().values()))
        self.nc.all_engine_barrier()

    _tile.TileContext._drain_and_barrier = _patched
    _tile.TileContext._ant_drain_patched = True


def _build(num_layers):
    import concourse.bass as bass
    import concourse.mybir as mybir
    from concourse.tile import TileContext

    _patch_tile_drain()
    f32 = mybir.dt.float32
    bf16 = mybir.dt.bfloat16
    SIG = mybir.ActivationFunctionType.Sigmoid
    TANH = mybir.ActivationFunctionType.Tanh

    nc = bass.Bass()
    d_h0 = nc.dram_tensor("h0b", [N, D], bf16, kind="ExternalInput")
    d_c0 = nc.dram_tensor("c0", [N, D], f32, kind="ExternalInput")
    d_ain = nc.dram_tensor("ainT", [N, N], bf16, kind="ExternalInput")
    d_aout = nc.dram_tensor("aoutT", [N, N], bf16, kind="ExternalInput")
    d_prex = nc.dram_tensor("preX", [N, 4 * D], bf16, kind="ExternalInput")
    d_uin = nc.dram_tensor("uin", [D, 4 * D], bf16, kind="ExternalInput")
    d_uout = nc.dram_tensor("uout", [D, 4 * D], bf16, kind="ExternalInput")
    d_nmask = nc.dram_tensor("nmask", [128, NT], f32, kind="ExternalInput")
    d_out = nc.dram_tensor("hout", [N, D], f32, kind="ExternalOutput")

    def row_tile(t, i):
        return t[i * 128 : (i + 1) * 128, :]

    with TileContext(nc) as tc:
        with (
            tc.tile_pool(name="persist", bufs=1) as pp,
            tc.tile_pool(name="gates", bufs=3) as gp,
            tc.tile_pool(name="tmp", bufs=6) as tp,
            tc.tile_pool(name="outp", bufs=3) as op,
            tc.tile_pool(name="gpsum", bufs=4, space="PSUM") as gps,
            tc.tile_pool(name="ppsum", bufs=4, space="PSUM") as pps,
        ):
            h_a = pp.tile([128, NT * D], bf16, tag="h_a")
            h_b = pp.tile([128, NT * D], bf16, tag="h_b")
            c_sb = pp.tile([128, NT * D], f32, tag="c_sb")
            a_in = pp.tile([128, NT * N], bf16, tag="a_in")
            a_out = pp.tile([128, NT * N], bf16, tag="a_out")
            prex = pp.tile([128, NT * 4 * D], bf16, tag="prex")
            uin = pp.tile([128, DT * 4 * D], bf16, tag="uin")
            uout = pp.tile([128, DT * 4 * D], bf16, tag="uout")
            hinT = pp.tile([128, DT * N], bf16, tag="hinT")
            houtT = pp.tile([128, DT * N], bf16, tag="houtT")
            nmask = pp.tile([128, NT], f32, tag="nmask")
            nc.sync.dma_start(out=nmask[:, :], in_=d_nmask[:, :])

            # input DMAs, chunked by tile so compute can start early
            for mt in range(NT):
                nc.sync.dma_start(
                    out=h_a[:, mt * D : (mt + 1) * D], in_=row_tile(d_h0, mt)
                )
            for mt in range(NT):
                nc.sync.dma_start(
                    out=a_in[:, mt * N : (mt + 1) * N], in_=row_tile(d_ain, mt)
                )
                nc.sync.dma_start(
                    out=a_out[:, mt * N : (mt + 1) * N], in_=row_tile(d_aout, mt)
                )
            for kt in range(DT):
                nc.sync.dma_start(
                    out=uin[:, kt * 4 * D : (kt + 1) * 4 * D], in_=row_tile(d_uin, kt)
                )
                nc.sync.dma_start(
                    out=uout[:, kt * 4 * D : (kt + 1) * 4 * D], in_=row_tile(d_uout, kt)
                )
            for nt in range(NT):
                nc.sync.dma_start(
                    out=prex[:, nt * 4 * D : (nt + 1) * 4 * D], in_=row_tile(d_prex, nt)
                )
                nc.sync.dma_start(
                    out=c_sb[:, nt * D : (nt + 1) * D], in_=row_tile(d_c0, nt)
                )

            h_src, h_dst = h_a, h_b
            for layer in range(num_layers):
                last = layer == num_layers - 1
                # ---- gather phase: h_inT/h_outT[d, n] = sum_m h[m,d] * A_T[m,n]
                for dt in range(DT):
                    for gout, a_sb in ((hinT, a_in), (houtT, a_out)):
                        ps0 = gps.tile([128, 512], f32, tag="gps")
                        ps1 = gps.tile([128, 512], f32, tag="gps")
                        for mt in range(NT):
                            lhs = h_src[:, mt * D + dt * 128 : mt * D + dt * 128 + 128]
                            nc.tensor.matmul(
                                ps0[:, :],
                                lhs,
                                a_sb[:, mt * N : mt * N + 512],
                                start=(mt == 0),
                                stop=(mt == NT - 1),
                            )
                            nc.tensor.matmul(
                                ps1[:, :],
                                lhs,
                                a_sb[:, mt * N + 512 : mt * N + 1024],
                                start=(mt == 0),
                                stop=(mt == NT - 1),
                            )
                        nc.vector.tensor_copy(
                            out=gout[:, dt * N : dt * N + 512], in_=ps0[:, :]
                        )
                        nc.vector.tensor_copy(
                            out=gout[:, dt * N + 512 : dt * N + 1024], in_=ps1[:, :]
                        )
                # ---- per node-tile: U matmuls + gates + state update
                for nt in range(NT):
                    pre_sb = gp.tile([128, 4 * D], f32, tag="pre_sb")
                    for eh in range(2):
                        pr = pps.tile([128, 512], f32, tag="pps")
                        acc = 0
                        for gT, u_sb in ((hinT, uin), (houtT, uout)):
                            for kt in range(DT):
                                nc.tensor.matmul(
                                    pr[:, :],
                                    gT[:, kt * N + nt * 128 : kt * N + nt * 128 + 128],
                                    u_sb[:, kt * 4 * D + eh * 512 : kt * 4 * D + eh * 512 + 512],
                                    start=(acc == 0),
                                    stop=(acc == 2 * DT - 1),
                                )
                                acc += 1
                        nc.vector.tensor_add(
                            out=pre_sb[:, eh * 512 : (eh + 1) * 512],
                            in0=pr[:, :],
                            in1=prex[:, nt * 4 * D + eh * 512 : nt * 4 * D + eh * 512 + 512],
                        )
                    gsig = gp.tile([128, 3 * D], f32, tag="gsig")
                    gtan = gp.tile([128, D], f32, tag="gtan")
                    nc.scalar.activation(gsig[:, :], pre_sb[:, 0 : 3 * D], SIG)
                    nc.scalar.activation(gtan[:, :], pre_sb[:, 3 * D : 4 * D], TANH)
                    cs = c_sb[:, nt * D : (nt + 1) * D]
                    t1 = tp.tile([128, D], f32, tag="t1")
                    t2 = tp.tile([128, D], f32, tag="t2")
                    nc.vector.tensor_mul(out=t1[:, :], in0=gsig[:, 2 * D : 3 * D], in1=cs)
                    nc.vector.tensor_mul(out=t2[:, :], in0=gsig[:, 0:D], in1=gtan[:, :])
                    nc.vector.tensor_add(out=cs, in0=t1[:, :], in1=t2[:, :])
                    tcn = tp.tile([128, D], f32, tag="tcn")
                    nc.scalar.activation(tcn[:, :], cs, TANH)
                    if last:
                        ho = op.tile([128, D], f32, tag="ho")
                        nc.vector.tensor_mul(
                            out=ho[:, :], in0=gsig[:, D : 2 * D], in1=tcn[:, :]
                        )
                        nc.sync.dma_start(
                            out=d_out[nt * 128 : (nt + 1) * 128, :], in_=ho[:, :]
                        )
                    else:
                        t3 = tp.tile([128, D], f32, tag="t3")
                        nc.vector.tensor_mul(
                            out=t3[:, :], in0=gsig[:, D : 2 * D], in1=tcn[:, :]
                        )
                        nc.vector.tensor_scalar_mul(
                            h_dst[:, nt * D : (nt + 1) * D],
                            t3[:, :],
                            nmask[:, nt : nt + 1],
                        )
                h_src, h_dst = h_dst, h_src
    return nc


def _host_prep(h0, c0, x_in, x_out, W_in, U_in, W_out, U_out, b,
               in_mask, out_mask, node_mask, in_nodes, out_nodes):
    bf = ml_dtypes.bfloat16
    f32 = np.float32
    # adjacency^T per sample, masked-source rows zeroed
    n_idx = np.broadcast_to(np.arange(N, dtype=np.int64)[:, None], (N, K))
    ains, aouts = [], []
    for bi in range(B):
        for (nodes, mask, store) in (
            (in_nodes[bi], in_mask[bi], ains),
            (out_nodes[bi], out_mask[bi], aouts),
        ):
            A = np.zeros((N, N), dtype=f32)
            np.add.at(A, (nodes.astype(np.int64).ravel(), n_idx.ravel()), mask.ravel())
            store.append(A.astype(bf))
    # layer-invariant x-side preactivation, gate-major columns [N, 4*D]
    Wi = np.transpose(W_in, (1, 0, 2)).reshape(D, 4 * D).astype(f32)
    Wo = np.transpose(W_out, (1, 0, 2)).reshape(D, 4 * D).astype(f32)
    bcat = b.reshape(4 * D).astype(f32)
    prex = (
        np.einsum("bnd,de->bne", x_in.astype(f32), Wi, optimize=True)
        + np.einsum("bnd,de->bne", x_out.astype(f32), Wo, optimize=True)
        + bcat[None, None, :]
    ).astype(f32)
    Ui = np.transpose(U_in, (1, 0, 2)).reshape(D, 4 * D).astype(bf)
    Uo = np.transpose(U_out, (1, 0, 2)).reshape(D, 4 * D).astype(bf)
    maps = []
    for bi in range(B):
        maps.append(
            {
                "h0b": h0[bi].astype(bf),
                "c0": c0[bi].astype(f32),
                "ainT": ains[bi],
                "aoutT": aouts[bi],
                "preX": np.ascontiguousarray(prex[bi]).astype(bf),
                "uin": Ui,
                "uout": Uo,
                "nmask": np.ascontiguousarray(
                    node_mask[bi].astype(f32).reshape(NT, 128).T
                ),
            }
        )
    return maps


def kernel(h0, c0, x_in, x_out, W_in, U_in, W_out, U_out, b,
           in_mask, out_mask, node_mask, in_nodes, out_nodes, num_layers,
           _trace=False):
    from concourse.bass_utils import run_bass_kernel_spmd

    h0, c0, x_in, x_out = (np.asarray(v, dtype=np.float32) for v in (h0, c0, x_in, x_out))
    W_in, U_in, W_out, U_out, b = (
        np.asarray(v, dtype=np.float32) for v in (W_in, U_in, W_out, U_out, b)
    )
    in_mask, out_mask, node_mask = (
        np.asarray(v, dtype=np.float32) for v in (in_mask, out_mask, node_mask)
    )
    in_nodes = np.asarray(in_nodes, dtype=np.int64)
    out_nodes = np.asarray(out_nodes, dtype=np.int64)
    L = int(num_layers)
    if L not in _CACHE:
        _CACHE[L] = _build(L)
    nc = _CACHE[L]
    in_maps = _host_prep(h0, c0, x_in, x_out, W_in, U_in, W_out, U_out, b,
                         in_mask, out_mask, node_mask, in_nodes, out_nodes)
    res = run_bass_kernel_spmd(nc, in_maps, list(range(B)), trace=_trace)
    out = np.stack([res.results[i]["hout"] for i in range(B)]).astype(np.float32)
    out *= np.asarray(node_mask, dtype=np.float32)[:, :, None]
    kernel._last_result = res
    return out



# revision 2
# speedup vs baseline: 6.7005x; 6.7005x over previous
"""Graph-LSTM (GsGLstm) Trainium2 kernel — transfer-optimized.

B=8 -> one sample per NeuronCore, pure data parallel. The axon tunnel
(~60-130MB/s h2d, ~35MB/s d2h) and the 1-CPU host dominate wall time, so
this version ships only raw data and does all preprocessing on device:

  - host ships per core: blob[4N,D] bf16 (h0|c0|x_in|x_out rows),
    idxm[N,2K] f32 (neighbor index, or -1 where the edge mask is 0),
    nmask[128,NT] f32. Weights ([4D,4D]+[1,4D] bf16, gate-major) are
    replicated, content-hashed, and cached on device across calls.
  - device builds the dense transposed adjacency from idxm with
    per-partition is_equal tensor_scalar ops against an iota row
    (A[n,m] = sum_k [idx[n,k]==m]), then DMA-transposes 128x128 blocks
    SBUF->SBUF into A_T[m,n] for the gather matmuls.
  - device computes pre_x = x_in@W_in + x_out@W_out + b (x transposed on
    load via DMA-transpose; b broadcast via a rank-1 ones matmul).
  - per layer: gather matmuls (h stationary, A_T moving) -> h_inT/h_outT
    [d,n] -> U matmuls -> +pre_x -> sigmoid/tanh -> c/h updates.
  - output h (node-masked on device) returns as bf16 and is widened on
    host.

The PJRT executable (shard_map over 8 cores) is traced/compiled once per
num_layers and cached, so steady-state calls pay only input transfer +
execute + output fetch.
"""

import numpy as np
import ml_dtypes
import hashlib

B, N, K, D = 8, 1024, 16, 256
NT = N // 128   # 8 node partition-tiles
DT = D // 128   # 2 feature partition-tiles
G4 = 4 * D      # 1024 gate-major preactivation columns

_RUNNERS = {}
_WCACHE = {}
BF16 = ml_dtypes.bfloat16


class _Result:
    """Shim matching BassKernelResults fields test.py touches."""

    def __init__(self, results=None, exec_time_ns=None, profile_json=None):
        self.results = results
        self.exec_time_ns = exec_time_ns
        self.profile_json = profile_json


def _patch_tile_drain():
    """walrus CTRL instructions have 2 sync-wait slots; TileContext's final
    drain can carry more and fails codegen. Split excess waits onto SP nops."""
    import concourse.tile as _tile

    if getattr(_tile.TileContext, "_ant_drain_patched", False):
        return
    ScopedClock = _tile.ScopedClock

    def _split_excess_waits(nc):
        import concourse.mybir as _mybir

        for f in nc.m.functions:
            for blk in f.blocks:
                insts = blk.instructions
                i = 0
                while i < len(insts):
                    ins = insts[i]
                    si = getattr(ins, "sync_info", None)
                    keep = 1
                    if si and si.on_wait and len(si.on_wait) > keep:
                        waits = list(si.on_wait)
                        head, tail = waits[:-keep], waits[-keep:]
                        si.on_wait.clear()
                        for w in tail:
                            si.on_wait.append(w)
                        eng = nc.engines[ins.engine]
                        pos = i
                        for w in head:
                            n = eng.nop(nofuse=True)
                            cur_list = nc.cur_bb.bb.instructions
                            assert cur_list[-1] is n.ins
                            cur_list.pop()
                            if n.ins.sync_info is None:
                                n.ins.sync_info = _mybir.SyncInfo(
                                    on_wait=[], on_update=[]
                                )
                            n.ins.sync_info.on_wait.append(w)
                            insts.insert(pos, n.ins)
                            pos += 1
                            i += 1
                    i += 1

    def _patched(self, tick_clock, wait_clock):
        drain_inst = self.nc.sync.drain()
        wait_clock.add_sem_waits(
            drain_inst.ins, ScopedClock({None: tick_clock.global_clock})
        )
        _split_excess_waits(self.nc)
        self.nc.all_engine_barrier()
        assert self.sems is not None
        popped = self.nc._tile_sem_poison_stack.pop()
        assert popped is self._sem_poison
        self.nc.clear_and_free_semaphores(list(self.sems.allocated().values()))
        self.nc.all_engine_barrier()

    _tile.TileContext._drain_and_barrier = _patched
    _tile.TileContext._ant_drain_patched = True


def _build(num_layers):
    import concourse.bass as bass
    import concourse.mybir as mybir
    from concourse.tile import TileContext

    _patch_tile_drain()
    f32 = mybir.dt.float32
    bf16 = mybir.dt.bfloat16
    SIG = mybir.ActivationFunctionType.Sigmoid
    TANH = mybir.ActivationFunctionType.Tanh
    EQ = mybir.AluOpType.is_equal

    nc = bass.Bass()
    d_blob = nc.dram_tensor("blob", [4 * N, D], bf16, kind="ExternalInput")
    d_idx = nc.dram_tensor("idxm", [N, 2 * K], f32, kind="ExternalInput")
    d_nm = nc.dram_tensor("nmask", [128, NT], f32, kind="ExternalInput")
    d_w = nc.dram_tensor("wcat", [4 * D, G4], bf16, kind="ExternalInput")
    d_b = nc.dram_tensor("bcat", [1, G4], bf16, kind="ExternalInput")
    d_out = nc.dram_tensor("hout", [N, D], bf16, kind="ExternalOutput")

    with TileContext(nc) as tc:
        with (
            tc.tile_pool(name="persist", bufs=1) as pp,
            tc.tile_pool(name="gates", bufs=2) as gp,
            tc.tile_pool(name="tmp", bufs=4) as tp,
            tc.tile_pool(name="eqp", bufs=4) as ep,
            tc.tile_pool(name="gpsum", bufs=4, space="PSUM") as gps,
            tc.tile_pool(name="ppsum", bufs=4, space="PSUM") as pps,
        ):
            h_a = pp.tile([128, NT * D], bf16, tag="h_a")
            h_b = pp.tile([128, NT * D], bf16, tag="h_b")
            c_sb = pp.tile([128, NT * D], f32, tag="c_sb")
            xT_in = pp.tile([128, DT * N], bf16, tag="xT_in")
            xT_out = pp.tile([128, DT * N], bf16, tag="xT_out")
            win = pp.tile([128, DT * G4], bf16, tag="win")
            wout = pp.tile([128, DT * G4], bf16, tag="wout")
            uin = pp.tile([128, DT * G4], bf16, tag="uin")
            uout = pp.tile([128, DT * G4], bf16, tag="uout")
            b_sb = pp.tile([1, G4], bf16, tag="b_sb")
            ones = pp.tile([1, 128], bf16, tag="ones")
            idxm_in = pp.tile([128, NT * K], f32, tag="idxm_in")
            idxm_out = pp.tile([128, NT * K], f32, tag="idxm_out")
            nmask = pp.tile([128, NT], f32, tag="nmask")
            iota_f = pp.tile([128, N], f32, tag="iota_f")
            A_in = pp.tile([128, NT * N], bf16, tag="A_in")
            A_out = pp.tile([128, NT * N], bf16, tag="A_out")
            AT_in = pp.tile([128, NT * N], bf16, tag="AT_in")
            AT_out = pp.tile([128, NT * N], bf16, tag="AT_out")
            hinT = pp.tile([128, DT * N], bf16, tag="hinT")
            houtT = pp.tile([128, DT * N], bf16, tag="houtT")
            prex = pp.tile([128, NT * G4], f32, tag="prex")

            # ---- input DMAs
            nc.sync.dma_start(out=nmask[:, :], in_=d_nm[:, :])
            nc.sync.dma_start(out=b_sb[:, :], in_=d_b[:, :])
            for mt in range(NT):
                nc.sync.dma_start(
                    out=h_a[:, mt * D : (mt + 1) * D],
                    in_=d_blob[mt * 128 : (mt + 1) * 128, :],
                )
            # c0 staged (bf16) into h_b, widened to f32 below
            for mt in range(NT):
                nc.sync.dma_start(
                    out=h_b[:, mt * D : (mt + 1) * D],
                    in_=d_blob[N + mt * 128 : N + (mt + 1) * 128, :],
                )
            for nt in range(NT):
                nc.sync.dma_start(
                    out=idxm_in[:, nt * K : (nt + 1) * K],
                    in_=d_idx[nt * 128 : (nt + 1) * 128, 0:K],
                )
                nc.sync.dma_start(
                    out=idxm_out[:, nt * K : (nt + 1) * K],
                    in_=d_idx[nt * 128 : (nt + 1) * 128, K : 2 * K],
                )
            # x_in / x_out loaded pre-transposed: [d, n] layout
            for xi, xT in ((2, xT_in), (3, xT_out)):
                for dt in range(DT):
                    nc.scalar.dma_start_transpose(
                        out=xT[:, dt * N : (dt + 1) * N],
                        in_=d_blob[xi * N : (xi + 1) * N, dt * 128 : (dt + 1) * 128],
                    )
            for w_sb, r0 in ((win, 0), (wout, D), (uin, 2 * D), (uout, 3 * D)):
                for kt in range(DT):
                    nc.sync.dma_start(
                        out=w_sb[:, kt * G4 : (kt + 1) * G4],
                        in_=d_w[r0 + kt * 128 : r0 + (kt + 1) * 128, :],
                    )
            nc.gpsimd.memset(ones[:, :], 1.0)
            nc.gpsimd.iota(
                iota_f[:, :],
                pattern=[[1, N]],
                base=0,
                channel_multiplier=0,
                allow_small_or_imprecise_dtypes=True,
            )
            nc.vector.tensor_copy(out=c_sb[:, :], in_=h_b[:, :])

            # ---- pre_x = b + x_in@W_in + x_out@W_out  (gate-major [n, 4D], f32)
            for nt in range(NT):
                for eh in range(2):
                    ps = pps.tile([128, 512], f32, tag="pps")
                    nc.tensor.matmul(
                        ps[:, :],
                        ones[:, :],
                        b_sb[:, eh * 512 : (eh + 1) * 512],
                        start=True,
                        stop=False,
                    )
                    acc = 0
                    for xT, w_sb in ((xT_in, win), (xT_out, wout)):
                        for dt in range(DT):
                            nc.tensor.matmul(
                                ps[:, :],
                                xT[:, dt * N + nt * 128 : dt * N + nt * 128 + 128],
                                w_sb[:, dt * G4 + eh * 512 : dt * G4 + eh * 512 + 512],
                                start=False,
                                stop=(acc == 2 * DT - 1),
                            )
                            acc += 1
                    nc.scalar.activation(
                        prex[:, nt * G4 + eh * 512 : nt * G4 + eh * 512 + 512],
                        ps[:, :],
                        mybir.ActivationFunctionType.Copy,
                    )

            # ---- adjacency build + transpose:
            # A[n, m] = sum_k [idxm[n,k] == m] (idxm = -1 where masked)
            for A_sb, AT_sb, idxm in (
                (A_in, AT_in, idxm_in),
                (A_out, AT_out, idxm_out),
            ):
                for nt in range(NT):
                    arow = A_sb[:, nt * N : (nt + 1) * N]
                    for k in range(K):
                        s = idxm[:, nt * K + k : nt * K + k + 1]
                        if k == 0:
                            nc.vector.tensor_scalar(
                                out=arow, in0=iota_f[:, :], scalar1=s,
                                scalar2=None, op0=EQ,
                            )
                        else:
                            eq = ep.tile([128, N], bf16, tag="eq")
                            nc.vector.tensor_scalar(
                                out=eq[:, :], in0=iota_f[:, :], scalar1=s,
                                scalar2=None, op0=EQ,
                            )
                            nc.gpsimd.tensor_add(out=arow, in0=arow, in1=eq[:, :])
                for nt in range(NT):
                    for mt in range(NT):
                        eng = nc.sync if (nt + mt) % 2 == 0 else nc.scalar
                        eng.dma_start_transpose(
                            out=AT_sb[:, mt * N + nt * 128 : mt * N + nt * 128 + 128],
                            in_=A_sb[:, nt * N + mt * 128 : nt * N + mt * 128 + 128],
                        )

            # ---- layers
            h_src, h_dst = h_a, h_b
            for layer in range(num_layers):
                last = layer == num_layers - 1
                # gather: h_inT/h_outT[d, n] = sum_m h[m, d] * A_T[m, n]
                for dt in range(DT):
                    for gout, a_sb in ((hinT, AT_in), (houtT, AT_out)):
                        ps0 = gps.tile([128, 512], f32, tag="gps")
                        ps1 = gps.tile([128, 512], f32, tag="gps")
                        for mt in range(NT):
                            lhs = h_src[:, mt * D + dt * 128 : mt * D + dt * 128 + 128]
                            nc.tensor.matmul(
                                ps0[:, :],
                                lhs,
                                a_sb[:, mt * N : mt * N + 512],
                                start=(mt == 0),
                                stop=(mt == NT - 1),
                            )
                            nc.tensor.matmul(
                                ps1[:, :],
                                lhs,
                                a_sb[:, mt * N + 512 : mt * N + 1024],
                                start=(mt == 0),
                                stop=(mt == NT - 1),
                            )
                        nc.vector.tensor_copy(
                            out=gout[:, dt * N : dt * N + 512], in_=ps0[:, :]
                        )
                        nc.vector.tensor_copy(
                            out=gout[:, dt * N + 512 : dt * N + 1024], in_=ps1[:, :]
                        )
                # per node-tile: U matmuls + gates + state update
                for nt in range(NT):
                    pre_sb = gp.tile([128, G4], f32, tag="pre_sb")
                    for eh in range(2):
                        pr = pps.tile([128, 512], f32, tag="pps")
                        acc = 0
                        for gT, u_sb in ((hinT, uin), (houtT, uout)):
                            for kt in range(DT):
                                nc.tensor.matmul(
                                    pr[:, :],
                                    gT[:, kt * N + nt * 128 : kt * N + nt * 128 + 128],
                                    u_sb[:, kt * G4 + eh * 512 : kt * G4 + eh * 512 + 512],
                                    start=(acc == 0),
                                    stop=(acc == 2 * DT - 1),
                                )
                                acc += 1
                        nc.vector.tensor_add(
                            out=pre_sb[:, eh * 512 : (eh + 1) * 512],
                            in0=pr[:, :],
                            in1=prex[:, nt * G4 + eh * 512 : nt * G4 + eh * 512 + 512],
                        )
                    gsig = gp.tile([128, 3 * D], f32, tag="gsig")
                    gtan = gp.tile([128, D], f32, tag="gtan")
                    nc.scalar.activation(gsig[:, :], pre_sb[:, 0 : 3 * D], SIG)
                    nc.scalar.activation(gtan[:, :], pre_sb[:, 3 * D : 4 * D], TANH)
                    cs = c_sb[:, nt * D : (nt + 1) * D]
                    t1 = tp.tile([128, D], f32, tag="t1")
                    t2 = tp.tile([128, D], f32, tag="t2")
                    nc.vector.tensor_mul(out=t1[:, :], in0=gsig[:, 2 * D : 3 * D], in1=cs)
                    nc.vector.tensor_mul(out=t2[:, :], in0=gsig[:, 0:D], in1=gtan[:, :])
                    nc.vector.tensor_add(out=cs, in0=t1[:, :], in1=t2[:, :])
                    tcn = tp.tile([128, D], f32, tag="tcn")
                    nc.scalar.activation(tcn[:, :], cs, TANH)
                    t3 = tp.tile([128, D], f32, tag="t3")
                    nc.vector.tensor_mul(
                        out=t3[:, :], in0=gsig[:, D : 2 * D], in1=tcn[:, :]
                    )
                    nc.vector.tensor_scalar_mul(
                        h_dst[:, nt * D : (nt + 1) * D],
                        t3[:, :],
                        nmask[:, nt : nt + 1],
                    )
                    if last:
                        nc.sync.dma_start(
                            out=d_out[nt * 128 : (nt + 1) * 128, :],
                            in_=h_dst[:, nt * D : (nt + 1) * D],
                        )
                h_src, h_dst = h_dst, h_src
    return nc


def _get_runner(L):
    if L in _RUNNERS:
        return _RUNNERS[L]
    import jax
    import jax.numpy as jnp
    from jax.sharding import Mesh, PartitionSpec, NamedSharding
    from jax.experimental.shard_map import shard_map
    from concourse import bass2jax, mybir

    nc = _build(L)
    bass2jax.install_neuronx_cc_hook()

    partition_name = nc.partition_id_tensor.name if nc.partition_id_tensor else None
    in_names, out_names, out_avals = [], [], []
    for alloc in nc.m.functions[0].allocations:
        if not isinstance(alloc, mybir.MemoryLocationSet):
            continue
        name = alloc.memorylocations[0].name
        if alloc.kind == "ExternalInput":
            if name != partition_name:
                in_names.append(name)
        elif alloc.kind == "ExternalOutput":
            shape = tuple(alloc.tensor_shape)
            dtype = mybir.dt.np(alloc.dtype)
            out_names.append(name)
            out_avals.append(jax.core.ShapedArray(shape, dtype))
    n_params = len(in_names)
    n_outs = len(out_avals)
    in_names_all = list(in_names) + list(out_names)
    if partition_name is not None:
        in_names_all.append(partition_name)
    donate = tuple(range(n_params, n_params + n_outs))

    def _body(*args):
        operands = list(args)
        if partition_name is not None:
            operands.append(bass2jax.partition_id_tensor())
        outs = bass2jax._bass_exec_p.bind(
            *operands,
            out_avals=tuple(out_avals),
            in_names=tuple(in_names_all),
            out_names=tuple(out_names),
            lowering_input_output_aliases=(),
            sim_require_finite=True,
            sim_require_nnan=True,
            nc=nc,
        )
        return tuple(outs)

    devices = jax.devices()[:B]
    mesh = Mesh(np.asarray(devices), ("core",))
    repl = {"wcat", "bcat"}
    in_specs = tuple(
        PartitionSpec() if nm in repl else PartitionSpec("core") for nm in in_names
    ) + (PartitionSpec("core"),) * n_outs
    out_specs = (PartitionSpec("core"),) * n_outs
    sharded = jax.jit(
        shard_map(
            _body, mesh=mesh, in_specs=in_specs, out_specs=out_specs, check_rep=False
        ),
        donate_argnums=donate,
        keep_unused=True,
    )
    zsh = NamedSharding(mesh, PartitionSpec("core"))
    zjit = jax.jit(
        lambda: jnp.zeros((B * N, D), jnp.bfloat16), out_shardings=zsh
    )
    wsh = NamedSharding(mesh, PartitionSpec())
    r = {
        "nc": nc,
        "in_names": in_names,
        "out_names": out_names,
        "fn": sharded,
        "zjit": zjit,
        "wsh": wsh,
        "jax": jax,
    }
    _RUNNERS[L] = r
    return r


def _prep_weights(r, W_in, U_in, W_out, U_out, b):
    import jax

    h = hashlib.blake2b(digest_size=16)
    for a in (W_in, U_in, W_out, U_out, b):
        h.update(a.tobytes())
    key = h.digest()
    if key in _WCACHE:
        return _WCACHE[key]
    wcat = np.empty((4 * D, G4), dtype=BF16)
    for i, W in enumerate((W_in, W_out, U_in, U_out)):
        # rows [iD:(i+1)D] = gate-major [D, 4D] view of W[g, d, e]
        wcat[i * D : (i + 1) * D, :] = np.transpose(W, (1, 0, 2)).reshape(D, G4)
    bcat = b.reshape(1, G4).astype(BF16)
    wdev = jax.device_put(wcat, r["wsh"])
    bdev = jax.device_put(bcat, r["wsh"])
    jax.block_until_ready([wdev, bdev])
    _WCACHE[key] = (wdev, bdev)
    return _WCACHE[key]


def _host_pack(h0, c0, x_in, x_out, in_mask, out_mask, node_mask,
               in_nodes, out_nodes):
    blob = np.empty((B, 4, N, D), dtype=BF16)
    blob[:, 0] = h0
    blob[:, 1] = c0
    blob[:, 2] = x_in
    blob[:, 3] = x_out
    idxm = np.empty((B, N, 2 * K), dtype=np.float32)
    np.copyto(idxm[:, :, :K], in_nodes)
    idxm[:, :, :K][in_mask == 0] = -1.0
    np.copyto(idxm[:, :, K:], out_nodes)
    idxm[:, :, K:][out_mask == 0] = -1.0
    nm = np.ascontiguousarray(
        node_mask.reshape(B, NT, 128).transpose(0, 2, 1)
    ).reshape(B * 128, NT)
    return blob.reshape(4 * B * N, D), idxm.reshape(B * N, 2 * K), nm


def kernel(h0, c0, x_in, x_out, W_in, U_in, W_out, U_out, b,
           in_mask, out_mask, node_mask, in_nodes, out_nodes, num_layers,
           _trace=False):
    h0, c0, x_in, x_out = (np.asarray(v, dtype=np.float32) for v in (h0, c0, x_in, x_out))
    W_in, U_in, W_out, U_out, b = (
        np.asarray(v, dtype=np.float32) for v in (W_in, U_in, W_out, U_out, b)
    )
    in_mask, out_mask, node_mask = (
        np.asarray(v, dtype=np.float32) for v in (in_mask, out_mask, node_mask)
    )
    in_nodes = np.asarray(in_nodes, dtype=np.int32)
    out_nodes = np.asarray(out_nodes, dtype=np.int32)
    L = int(num_layers)

    r = _get_runner(L)
    blob, idxm, nm = _host_pack(h0, c0, x_in, x_out, in_mask, out_mask,
                                node_mask, in_nodes, out_nodes)
    wdev, bdev = _prep_weights(r, W_in, U_in, W_out, U_out, b)

    if _trace:
        # diagnostic path: per-core in_maps through the stock spmd runner
        from concourse.bass_utils import run_bass_kernel_spmd

        maps = []
        for bi in range(B):
            maps.append({
                "blob": np.ascontiguousarray(
                    blob.reshape(B, 4 * N, D)[bi]),
                "idxm": np.ascontiguousarray(idxm.reshape(B, N, 2 * K)[bi]),
                "nmask": np.ascontiguousarray(nm.reshape(B, 128, NT)[bi]),
                "wcat": np.asarray(wdev),
                "bcat": np.asarray(bdev),
            })
        res = run_bass_kernel_spmd(r["nc"], maps, list(range(B)), trace=True)
        out = np.stack([
            np.asarray(res.results[i]["hout"]).astype(np.float32)
            for i in range(B)
        ])
        kernel._last_result = res
        return out

    args = []
    by_name = {"blob": blob, "idxm": idxm, "nmask": nm, "wcat": wdev, "bcat": bdev}
    for nm_ in r["in_names"]:
        args.append(by_name[nm_])
    zeros = r["zjit"]()
    out_arrs = r["fn"](*args, zeros)
    out = np.asarray(out_arrs[0]).reshape(B, N, D).astype(np.float32)
    kernel._last_result = _Result(
        results=[{"hout": out[i]} for i in range(B)]
    )
    return out


# revision 5
# speedup vs baseline: 7.6886x; 1.1475x over previous
"""Graph-LSTM (GsGLstm) Trainium2 kernel — transfer-optimized.

B=8 -> one sample per NeuronCore, pure data parallel. The axon tunnel
(~60-130MB/s h2d, ~35MB/s d2h) and the 1-CPU host dominate wall time, so
this version ships only raw data and does all preprocessing on device:

  - host ships per core: blob[4N,D] bf16 (h0|c0|x_in|x_out rows),
    idxm[N,2K] f32 (neighbor index, or -1 where the edge mask is 0),
    nmask[128,NT] f32. Weights ([4D,4D]+[1,4D] bf16, gate-major) are
    replicated, content-hashed, and cached on device across calls.
  - device builds the dense transposed adjacency from idxm with
    per-partition is_equal tensor_scalar ops against an iota row
    (A[n,m] = sum_k [idx[n,k]==m]), then DMA-transposes 128x128 blocks
    SBUF->SBUF into A_T[m,n] for the gather matmuls.
  - device computes pre_x = x_in@W_in + x_out@W_out + b (x transposed on
    load via DMA-transpose; b broadcast via a rank-1 ones matmul).
  - per layer: gather matmuls (h stationary, A_T moving) -> h_inT/h_outT
    [d,n] -> U matmuls -> +pre_x -> sigmoid/tanh -> c/h updates.
  - output h (node-masked on device) returns as bf16 and is widened on
    host.

The PJRT executable (shard_map over 8 cores) is traced/compiled once per
num_layers and cached, so steady-state calls pay only input transfer +
execute + output fetch.
"""

import numpy as np
import ml_dtypes
import hashlib

B, N, K, D = 8, 1024, 16, 256
NT = N // 128   # 8 node partition-tiles
DT = D // 128   # 2 feature partition-tiles
G4 = 4 * D      # 1024 gate-major preactivation columns

_RUNNERS = {}
_WCACHE = {}
BF16 = ml_dtypes.bfloat16


class _Result:
    """Shim matching BassKernelResults fields test.py touches."""

    def __init__(self, results=None, exec_time_ns=None, profile_json=None):
        self.results = results
        self.exec_time_ns = exec_time_ns
        self.profile_json = profile_json


def _patch_tile_drain():
    """walrus CTRL instructions have 2 sync-wait slots; TileContext's final
    drain can carry more and fails codegen. Split excess waits onto SP nops."""
    import concourse.tile as _tile

    if getattr(_tile.TileContext, "_ant_drain_patched", False):
        return
    ScopedClock = _tile.ScopedClock

    def _split_excess_waits(nc):
        import concourse.mybir as _mybir

        for f in nc.m.functions:
            for blk in f.blocks:
                insts = blk.instructions
                i = 0
                while i < len(insts):
                    ins = insts[i]
                    si = getattr(ins, "sync_info", None)
                    keep = 1
                    if si and si.on_wait and len(si.on_wait) > keep:
                        waits = list(si.on_wait)
                        head, tail = waits[:-keep], waits[-keep:]
                        si.on_wait.clear()
                        for w in tail:
                            si.on_wait.append(w)
                        eng = nc.engines[ins.engine]
                        pos = i
                        for w in head:
                            n = eng.nop(nofuse=True)
                            cur_list = nc.cur_bb.bb.instructions
                            assert cur_list[-1] is n.ins
                            cur_list.pop()
                            if n.ins.sync_info is None:
                                n.ins.sync_info = _mybir.SyncInfo(
                                    on_wait=[], on_update=[]
                                )
                            n.ins.sync_info.on_wait.append(w)
                            insts.insert(pos, n.ins)
                            pos += 1
                            i += 1
                    i += 1

    def _patched(self, tick_clock, wait_clock):
        drain_inst = self.nc.sync.drain()
        wait_clock.add_sem_waits(
            drain_inst.ins, ScopedClock({None: tick_clock.global_clock})
        )
        _split_excess_waits(self.nc)
        self.nc.all_engine_barrier()
        assert self.sems is not None
        popped = self.nc._tile_sem_poison_stack.pop()
        assert popped is self._sem_poison
        self.nc.clear_and_free_semaphores(list(self.sems.allocated().values()))
        self.nc.all_engine_barrier()

    _tile.TileContext._drain_and_barrier = _patched
    _tile.TileContext._ant_drain_patched = True


def _build(num_layers):
    import concourse.bass as bass
    import concourse.mybir as mybir
    from concourse.tile import TileContext

    _patch_tile_drain()
    f32 = mybir.dt.float32
    bf16 = mybir.dt.bfloat16
    SIG = mybir.ActivationFunctionType.Sigmoid
    TANH = mybir.ActivationFunctionType.Tanh
    EQ = mybir.AluOpType.is_equal

    nc = bass.Bass()
    d_blob = nc.dram_tensor("blob", [4 * N, D], bf16, kind="ExternalInput")
    d_idx = nc.dram_tensor("idxm", [N, 2 * K], f32, kind="ExternalInput")
    d_nm = nc.dram_tensor("nmask", [128, NT], f32, kind="ExternalInput")
    d_w = nc.dram_tensor("wcat", [4 * D, G4], bf16, kind="ExternalInput")
    d_b = nc.dram_tensor("bcat", [1, G4], bf16, kind="ExternalInput")
    d_out = nc.dram_tensor("hout", [N, D], bf16, kind="ExternalOutput")

    with TileContext(nc) as tc:
        with (
            tc.tile_pool(name="persist", bufs=1) as pp,
            tc.tile_pool(name="gates", bufs=2) as gp,
            tc.tile_pool(name="tmp", bufs=4) as tp,
            tc.tile_pool(name="eqp", bufs=4) as ep,
            tc.tile_pool(name="gpsum", bufs=4, space="PSUM") as gps,
            tc.tile_pool(name="ppsum", bufs=4, space="PSUM") as pps,
        ):
            h_a = pp.tile([128, NT * D], bf16, tag="h_a")
            h_b = pp.tile([128, NT * D], bf16, tag="h_b")
            c_sb = pp.tile([128, NT * D], f32, tag="c_sb")
            xT_in = pp.tile([128, DT * N], bf16, tag="xT_in")
            xT_out = pp.tile([128, DT * N], bf16, tag="xT_out")
            win = pp.tile([128, DT * G4], bf16, tag="win")
            wout = pp.tile([128, DT * G4], bf16, tag="wout")
            uin = pp.tile([128, DT * G4], bf16, tag="uin")
            uout = pp.tile([128, DT * G4], bf16, tag="uout")
            b_sb = pp.tile([1, G4], bf16, tag="b_sb")
            ones = pp.tile([1, 128], bf16, tag="ones")
            idxm_in = pp.tile([128, NT * K], f32, tag="idxm_in")
            idxm_out = pp.tile([128, NT * K], f32, tag="idxm_out")
            nmask = pp.tile([128, NT], f32, tag="nmask")
            iota_f = pp.tile([128, N], f32, tag="iota_f")
            A_in = pp.tile([128, NT * N], bf16, tag="A_in")
            A_out = pp.tile([128, NT * N], bf16, tag="A_out")
            AT_in = pp.tile([128, NT * N], bf16, tag="AT_in")
            AT_out = pp.tile([128, NT * N], bf16, tag="AT_out")
            hinT = pp.tile([128, DT * N], bf16, tag="hinT")
            houtT = pp.tile([128, DT * N], bf16, tag="houtT")
            prex = pp.tile([128, NT * G4], f32, tag="prex")

            # ---- input DMAs
            nc.sync.dma_start(out=nmask[:, :], in_=d_nm[:, :])
            nc.sync.dma_start(out=b_sb[:, :], in_=d_b[:, :])
            for mt in range(NT):
                nc.sync.dma_start(
                    out=h_a[:, mt * D : (mt + 1) * D],
                    in_=d_blob[mt * 128 : (mt + 1) * 128, :],
                )
            # c0 staged (bf16) into h_b, widened to f32 below
            for mt in range(NT):
                nc.sync.dma_start(
                    out=h_b[:, mt * D : (mt + 1) * D],
                    in_=d_blob[N + mt * 128 : N + (mt + 1) * 128, :],
                )
            for nt in range(NT):
                nc.sync.dma_start(
                    out=idxm_in[:, nt * K : (nt + 1) * K],
                    in_=d_idx[nt * 128 : (nt + 1) * 128, 0:K],
                )
                nc.sync.dma_start(
                    out=idxm_out[:, nt * K : (nt + 1) * K],
                    in_=d_idx[nt * 128 : (nt + 1) * 128, K : 2 * K],
                )
            # x_in / x_out loaded pre-transposed: [d, n] layout
            for xi, xT in ((2, xT_in), (3, xT_out)):
                for dt in range(DT):
                    nc.scalar.dma_start_transpose(
                        out=xT[:, dt * N : (dt + 1) * N],
                        in_=d_blob[xi * N : (xi + 1) * N, dt * 128 : (dt + 1) * 128],
                    )
            for w_sb, r0 in ((win, 0), (wout, D), (uin, 2 * D), (uout, 3 * D)):
                for kt in range(DT):
                    nc.sync.dma_start(
                        out=w_sb[:, kt * G4 : (kt + 1) * G4],
                        in_=d_w[r0 + kt * 128 : r0 + (kt + 1) * 128, :],
                    )
            nc.gpsimd.memset(ones[:, :], 1.0)
            nc.gpsimd.iota(
                iota_f[:, :],
                pattern=[[1, N]],
                base=0,
                channel_multiplier=0,
                allow_small_or_imprecise_dtypes=True,
            )
            nc.vector.tensor_copy(out=c_sb[:, :], in_=h_b[:, :])

            # ---- pre_x = b + x_in@W_in + x_out@W_out  (gate-major [n, 4D], f32)
            for nt in range(NT):
                for eh in range(2):
                    ps = pps.tile([128, 512], f32, tag="pps")
                    nc.tensor.matmul(
                        ps[:, :],
                        ones[:, :],
                        b_sb[:, eh * 512 : (eh + 1) * 512],
                        start=True,
                        stop=False,
                    )
                    acc = 0
                    for xT, w_sb in ((xT_in, win), (xT_out, wout)):
                        for dt in range(DT):
                            nc.tensor.matmul(
                                ps[:, :],
                                xT[:, dt * N + nt * 128 : dt * N + nt * 128 + 128],
                                w_sb[:, dt * G4 + eh * 512 : dt * G4 + eh * 512 + 512],
                                start=False,
                                stop=(acc == 2 * DT - 1),
                            )
                            acc += 1
                    nc.scalar.activation(
                        prex[:, nt * G4 + eh * 512 : nt * G4 + eh * 512 + 512],
                        ps[:, :],
                        mybir.ActivationFunctionType.Copy,
                    )

            # ---- adjacency build + transpose:
            # A[n, m] = sum_k [idxm[n,k] == m] (idxm = -1 where masked)
            for A_sb, AT_sb, idxm in (
                (A_in, AT_in, idxm_in),
                (A_out, AT_out, idxm_out),
            ):
                for nt in range(NT):
                    arow = A_sb[:, nt * N : (nt + 1) * N]
                    for k in range(K):
                        s = idxm[:, nt * K + k : nt * K + k + 1]
                        if k == 0:
                            nc.vector.tensor_scalar(
                                out=arow, in0=iota_f[:, :], scalar1=s,
                                scalar2=None, op0=EQ,
                            )
                        else:
                            eq = ep.tile([128, N], bf16, tag="eq")
                            nc.vector.tensor_scalar(
                                out=eq[:, :], in0=iota_f[:, :], scalar1=s,
                                scalar2=None, op0=EQ,
                            )
                            nc.gpsimd.tensor_add(out=arow, in0=arow, in1=eq[:, :])
                for nt in range(NT):
                    for mt in range(NT):
                        eng = nc.sync if (nt + mt) % 2 == 0 else nc.scalar
                        eng.dma_start_transpose(
                            out=AT_sb[:, mt * N + nt * 128 : mt * N + nt * 128 + 128],
                            in_=A_sb[:, nt * N + mt * 128 : nt * N + mt * 128 + 128],
                        )

            # ---- layers
            h_src, h_dst = h_a, h_b
            for layer in range(num_layers):
                last = layer == num_layers - 1
                # gather: h_inT/h_outT[d, n] = sum_m h[m, d] * A_T[m, n]
                for dt in range(DT):
                    for gout, a_sb in ((hinT, AT_in), (houtT, AT_out)):
                        ps0 = gps.tile([128, 512], f32, tag="gps")
                        ps1 = gps.tile([128, 512], f32, tag="gps")
                        for mt in range(NT):
                            lhs = h_src[:, mt * D + dt * 128 : mt * D + dt * 128 + 128]
                            nc.tensor.matmul(
                                ps0[:, :],
                                lhs,
                                a_sb[:, mt * N : mt * N + 512],
                                start=(mt == 0),
                                stop=(mt == NT - 1),
                            )
                            nc.tensor.matmul(
                                ps1[:, :],
                                lhs,
                                a_sb[:, mt * N + 512 : mt * N + 1024],
                                start=(mt == 0),
                                stop=(mt == NT - 1),
                            )
                        nc.vector.tensor_copy(
                            out=gout[:, dt * N : dt * N + 512], in_=ps0[:, :]
                        )
                        nc.vector.tensor_copy(
                            out=gout[:, dt * N + 512 : dt * N + 1024], in_=ps1[:, :]
                        )
                # per node-tile: U matmuls + gates + state update
                for nt in range(NT):
                    pre_sb = gp.tile([128, G4], f32, tag="pre_sb")
                    for eh in range(2):
                        pr = pps.tile([128, 512], f32, tag="pps")
                        acc = 0
                        for gT, u_sb in ((hinT, uin), (houtT, uout)):
                            for kt in range(DT):
                                nc.tensor.matmul(
                                    pr[:, :],
                                    gT[:, kt * N + nt * 128 : kt * N + nt * 128 + 128],
                                    u_sb[:, kt * G4 + eh * 512 : kt * G4 + eh * 512 + 512],
                                    start=(acc == 0),
                                    stop=(acc == 2 * DT - 1),
                                )
                                acc += 1
                        nc.vector.tensor_add(
                            out=pre_sb[:, eh * 512 : (eh + 1) * 512],
                            in0=pr[:, :],
                            in1=prex[:, nt * G4 + eh * 512 : nt * G4 + eh * 512 + 512],
                        )
                    gsig = gp.tile([128, 3 * D], f32, tag="gsig")
                    gtan = gp.tile([128, D], f32, tag="gtan")
                    nc.scalar.activation(gsig[:, :], pre_sb[:, 0 : 3 * D], SIG)
                    nc.scalar.activation(gtan[:, :], pre_sb[:, 3 * D : 4 * D], TANH)
                    cs = c_sb[:, nt * D : (nt + 1) * D]
                    t1 = tp.tile([128, D], f32, tag="t1")
                    t2 = tp.tile([128, D], f32, tag="t2")
                    nc.vector.tensor_mul(out=t1[:, :], in0=gsig[:, 2 * D : 3 * D], in1=cs)
                    nc.vector.tensor_mul(out=t2[:, :], in0=gsig[:, 0:D], in1=gtan[:, :])
                    nc.vector.tensor_add(out=cs, in0=t1[:, :], in1=t2[:, :])
                    tcn = tp.tile([128, D], f32, tag="tcn")
                    nc.scalar.activation(tcn[:, :], cs, TANH)
                    t3 = tp.tile([128, D], f32, tag="t3")
                    nc.vector.tensor_mul(
                        out=t3[:, :], in0=gsig[:, D : 2 * D], in1=tcn[:, :]
                    )
                    nc.vector.tensor_scalar_mul(
                        h_dst[:, nt * D : (nt + 1) * D],
                        t3[:, :],
                        nmask[:, nt : nt + 1],
                    )
                    if last:
                        nc.sync.dma_start(
                            out=d_out[nt * 128 : (nt + 1) * 128, :],
                            in_=h_dst[:, nt * D : (nt + 1) * D],
                        )
                h_src, h_dst = h_dst, h_src
    return nc


def _get_runner(L):
    if L in _RUNNERS:
        return _RUNNERS[L]
    import jax
    import jax.numpy as jnp
    from jax.sharding import Mesh, PartitionSpec, NamedSharding
    from jax.experimental.shard_map import shard_map
    from concourse import bass2jax, mybir

    nc = _build(L)
    bass2jax.install_neuronx_cc_hook()

    partition_name = nc.partition_id_tensor.name if nc.partition_id_tensor else None
    in_names, out_names, out_avals = [], [], []
    for alloc in nc.m.functions[0].allocations:
        if not isinstance(alloc, mybir.MemoryLocationSet):
            continue
        name = alloc.memorylocations[0].name
        if alloc.kind == "ExternalInput":
            if name != partition_name:
                in_names.append(name)
        elif alloc.kind == "ExternalOutput":
            shape = tuple(alloc.tensor_shape)
            dtype = mybir.dt.np(alloc.dtype)
            out_names.append(name)
            out_avals.append(jax.core.ShapedArray(shape, dtype))
    n_outs = len(out_avals)
    in_names_all = list(in_names) + list(out_names)
    if partition_name is not None:
        in_names_all.append(partition_name)

    def _body(*args):
        operands = list(args)
        if partition_name is not None:
            operands.append(bass2jax.partition_id_tensor())
        outs = bass2jax._bass_exec_p.bind(
            *operands,
            out_avals=tuple(out_avals),
            in_names=tuple(in_names_all),
            out_names=tuple(out_names),
            lowering_input_output_aliases=(),
            sim_require_finite=True,
            sim_require_nnan=True,
            nc=nc,
        )
        return tuple(outs)

    devices = jax.devices()[:B]
    mesh = Mesh(np.asarray(devices), ("core",))
    repl = {"wcat", "bcat"}
    in_specs = tuple(
        PartitionSpec() if nm in repl else PartitionSpec("core") for nm in in_names
    ) + (PartitionSpec("core"),) * n_outs
    out_specs = (PartitionSpec("core"),) * n_outs
    # The kernel writes every byte of hout, so the pre-zeroed output
    # staging buffer's contents never matter: pass one persistent device
    # buffer each call instead of donating fresh zeros (saves a dispatch).
    sharded = jax.jit(
        shard_map(
            _body, mesh=mesh, in_specs=in_specs, out_specs=out_specs, check_rep=False
        ),
        keep_unused=True,
    )
    zsh = NamedSharding(mesh, PartitionSpec("core"))
    zbuf = jax.device_put(np.zeros((B * N, D), BF16), zsh)
    wsh = NamedSharding(mesh, PartitionSpec())
    r = {
        "nc": nc,
        "in_names": in_names,
        "out_names": out_names,
        "fn": sharded,
        "zbuf": zbuf,
        "wsh": wsh,
        "jax": jax,
    }
    _RUNNERS[L] = r
    return r


def _prep_weights(r, W_in, U_in, W_out, U_out, b):
    import jax

    h = hashlib.blake2b(digest_size=16)
    for a in (W_in, U_in, W_out, U_out, b):
        h.update(a.tobytes())
    key = h.digest()
    if key in _WCACHE:
        return _WCACHE[key]
    wcat = np.empty((4 * D, G4), dtype=BF16)
    for i, W in enumerate((W_in, W_out, U_in, U_out)):
        # rows [iD:(i+1)D] = gate-major [D, 4D] view of W[g, d, e]
        wcat[i * D : (i + 1) * D, :] = np.transpose(W, (1, 0, 2)).reshape(D, G4)
    bcat = b.reshape(1, G4).astype(BF16)
    wdev = jax.device_put(wcat, r["wsh"])
    bdev = jax.device_put(bcat, r["wsh"])
    jax.block_until_ready([wdev, bdev])
    _WCACHE[key] = (wdev, bdev)
    return _WCACHE[key]


def _host_pack(h0, c0, x_in, x_out, in_mask, out_mask, node_mask,
               in_nodes, out_nodes):
    blob = np.empty((B, 4, N, D), dtype=BF16)
    blob[:, 0] = h0
    blob[:, 1] = c0
    blob[:, 2] = x_in
    blob[:, 3] = x_out
    idxm = np.empty((B, N, 2 * K), dtype=np.float32)
    np.copyto(idxm[:, :, :K], in_nodes)
    idxm[:, :, :K][in_mask == 0] = -1.0
    np.copyto(idxm[:, :, K:], out_nodes)
    idxm[:, :, K:][out_mask == 0] = -1.0
    nm = np.ascontiguousarray(
        node_mask.reshape(B, NT, 128).transpose(0, 2, 1)
    ).reshape(B * 128, NT)
    return blob.reshape(4 * B * N, D), idxm.reshape(B * N, 2 * K), nm


def kernel(h0, c0, x_in, x_out, W_in, U_in, W_out, U_out, b,
           in_mask, out_mask, node_mask, in_nodes, out_nodes, num_layers,
           _trace=False):
    h0, c0, x_in, x_out = (np.asarray(v, dtype=np.float32) for v in (h0, c0, x_in, x_out))
    W_in, U_in, W_out, U_out, b = (
        np.asarray(v, dtype=np.float32) for v in (W_in, U_in, W_out, U_out, b)
    )
    in_mask, out_mask, node_mask = (
        np.asarray(v, dtype=np.float32) for v in (in_mask, out_mask, node_mask)
    )
    in_nodes = np.asarray(in_nodes, dtype=np.int32)
    out_nodes = np.asarray(out_nodes, dtype=np.int32)
    L = int(num_layers)

    r = _get_runner(L)
    blob, idxm, nm = _host_pack(h0, c0, x_in, x_out, in_mask, out_mask,
                                node_mask, in_nodes, out_nodes)
    wdev, bdev = _prep_weights(r, W_in, U_in, W_out, U_out, b)

    if _trace:
        # diagnostic path: per-core in_maps through the stock spmd runner
        from concourse.bass_utils import run_bass_kernel_spmd

        maps = []
        for bi in range(B):
            maps.append({
                "blob": np.ascontiguousarray(
                    blob.reshape(B, 4 * N, D)[bi]),
                "idxm": np.ascontiguousarray(idxm.reshape(B, N, 2 * K)[bi]),
                "nmask": np.ascontiguousarray(nm.reshape(B, 128, NT)[bi]),
                "wcat": np.asarray(wdev),
                "bcat": np.asarray(bdev),
            })
        res = run_bass_kernel_spmd(r["nc"], maps, list(range(B)), trace=True)
        out = np.stack([
            np.asarray(res.results[i]["hout"]).astype(np.float32)
            for i in range(B)
        ])
        kernel._last_result = res
        return out

    args = []
    by_name = {"blob": blob, "idxm": idxm, "nmask": nm, "wcat": wdev, "bcat": bdev}
    for nm_ in r["in_names"]:
        args.append(by_name[nm_])
    out_arrs = r["fn"](*args, r["zbuf"])
    out = np.asarray(out_arrs[0]).reshape(B, N, D).astype(np.float32)
    kernel._last_result = _Result(
        results=[{"hout": out[i]} for i in range(B)]
    )
    return out
